# revision 7
# baseline (speedup 1.0000x reference)
"""Multi-Head Latent Attention (MLA) prefill kernel for 8 Trainium2 NeuronCores.

Sharding: latent down-projections row-split 8 ways + AllGather (split kv/q for
overlap); up-projections and attention head-split (2 heads/core); AllToAll
converts head-split attention output to token-split for the output projection.

Precision: bf16 weights/activations end-to-end (measured 4.7e-3 rel err),
fp8e4 DoubleRow matmuls for attention scores (q/k packed nope+rope in a
256-deep contraction, 4x fewer PE cycles; measured 1.15e-2 rel err total);
P and V stay bf16 (fp8 there fails the 2e-2 budget). f32 PSUM throughout.
"""
import sys
if '/opt/trn_rl_repo' not in sys.path:
    sys.path.insert(0, '/opt/trn_rl_repo')

import math
import numpy as np
import ml_dtypes

import concourse.bass as bass
import concourse.tile as tile
import concourse.mybir as mybir
from concourse import bacc

F32 = mybir.dt.float32
F32R = mybir.dt.float32r
BF16 = mybir.dt.bfloat16
F8 = mybir.dt.float8e4
AF = mybir.ActivationFunctionType
ALU = mybir.AluOpType
DR = mybir.MatmulPerfMode.DoubleRow
BF = ml_dtypes.bfloat16

B, S, DIM, H = 2, 2048, 2048, 16
NOPE, ROPE, QKD, VD = 128, 64, 192, 128
QLR, KVLR = 512, 512
EPS = 1e-6
NC = 8
N = B * S              # 4096 flattened tokens
R = N // NC            # 512 tokens per core (phase 1 / output rows)
HPC = H // NC          # 2 heads per core
NBLK = N // R          # 8 token blocks (= AG shards)
SCALE = 1.0 / math.sqrt(QKD)
NEG = -10000.0         # additive mask value (exp(NEG*SCALE) == 0)

SKIP, PLAIN = -2, -1   # mask block classes (>=0 -> index into mask blocks)


def _rope_tables():
    freqs = (1.0 / (10000.0 ** (np.arange(0, ROPE, 2, dtype=np.float32) / ROPE)))
    ang = np.arange(S, dtype=np.float32)[:, None] * freqs[None, :]      # [S, 32]
    return np.cos(ang).T.copy(), np.sin(ang).T.copy()                   # [32, S]


def _classify_mask(mask):
    """Per (q-chunk of 512, k-block of 128): SKIP / PLAIN / index into deduped
    additive mask blocks (0 where allowed, NEG where masked), [128 k, 512 q]."""
    cls = [[PLAIN] * (S // 128) for _ in range(S // 512)]
    blocks, keys = [], {}
    for qc in range(S // 512):
        sub_q = mask[qc * 512:(qc + 1) * 512]
        for kb in range(S // 128):
            blk = np.asarray(sub_q[:, kb * 128:(kb + 1) * 128])
            if not blk.any():
                cls[qc][kb] = SKIP
            elif blk.all():
                cls[qc][kb] = PLAIN
            else:
                key = blk.tobytes()
                if key not in keys:
                    keys[key] = len(blocks)
                    blocks.append(np.where(blk.T, 0.0, NEG).astype(BF))
                cls[qc][kb] = keys[key]
    blocks = (np.stack(blocks) if blocks
              else np.zeros((1, 128, 512), BF))
    return cls, blocks


def _build(cls, nmask, flags, repeat=1, sim_mode=False):
    """Emit the bass program. cls/nmask/flags are compile-time schedule data."""
    nc = bacc.Bacc(None, num_devices=NC)

    LAT = KVLR + ROPE  # 576

    # ---- I/O ----
    x_c = nc.dram_tensor("x_c", [128, DIM // 128, R], BF16, kind="ExternalInput")
    wqaT = nc.dram_tensor("wqaT", [4, 128, DIM // 128, 128], BF16, kind="ExternalInput")
    wkvaT = nc.dram_tensor("wkvaT", [5, 128, DIM // 128, 128], BF16, kind="ExternalInput")
    bqa = nc.dram_tensor("bqa", [QLR], F32, kind="ExternalInput")
    bkva = nc.dram_tensor("bkva", [KVLR + ROPE], F32, kind="ExternalInput")
    qnw = nc.dram_tensor("qnw", [QLR], F32, kind="ExternalInput")
    kvnw = nc.dram_tensor("kvnw", [KVLR], F32, kind="ExternalInput")
    # trig tables for split-halves rope: t1 = [c;c], t2 = [s;-s]
    t1c_d = nc.dram_tensor("t1c", [64, R], BF16, kind="ExternalInput")
    t2c_d = nc.dram_tensor("t2c", [64, R], BF16, kind="ExternalInput")
    t1q_d = nc.dram_tensor("t1q", [128, S], BF16, kind="ExternalInput")
    t2q_d = nc.dram_tensor("t2q", [128, S], BF16, kind="ExternalInput")
    smat_d = nc.dram_tensor("smat", [128, 128], F32, kind="ExternalInput")
    wqbT = nc.dram_tensor("wqbT", [128, 3, 4, 128], BF16, kind="ExternalInput")
    bqb = nc.dram_tensor("bqb", [HPC * QKD], F32, kind="ExternalInput")
    wkbT = nc.dram_tensor("wkbT", [128, 2, 4, 128], BF16, kind="ExternalInput")
    bkb = nc.dram_tensor("bkb", [HPC * NOPE], F32, kind="ExternalInput")
    wvbT = nc.dram_tensor("wvbT", [128, 4, HPC * VD], BF16, kind="ExternalInput")
    bvb = nc.dram_tensor("bvb", [1, HPC * VD], F32, kind="ExternalInput")
    woT = nc.dram_tensor("woT", [4, 128, 16, 512], BF16, kind="ExternalInput")
    wob = nc.dram_tensor("wob", [1, DIM], F32, kind="ExternalInput")
    maskblk = nc.dram_tensor("maskblk", [max(nmask, 1), 128, 512], BF16,
                             kind="ExternalInput")
    out_c = nc.dram_tensor("out", [R, DIM], F32, kind="ExternalOutput")

    with tile.TileContext(nc) as tc:
        with tc.tile_pool(name="konst", bufs=1) as konst, \
             tc.tile_pool(name="dram", bufs=1, space="DRAM") as dram:

            # ---- constants ----
            ones_f = konst.tile([128, 1], F32)
            nc.vector.memset(ones_f[:, :], 1.0)
            ones_col = konst.tile([128, 1], F32R)
            nc.vector.tensor_copy(out=ones_col[:, :], in_=ones_f[:, :])
            ones_bcol = konst.tile([128, 1], BF16)
            nc.vector.memset(ones_bcol[:, :], 1.0)
            ones_rf = konst.tile([1, 128], F32)
            nc.vector.memset(ones_rf[:, :], 1.0)
            ones_row = konst.tile([1, 128], F32R)
            nc.vector.tensor_copy(out=ones_row[:, :], in_=ones_rf[:, :])
            eps_t = konst.tile([1, 1], F32)
            nc.vector.memset(eps_t[:, :], EPS)
            smat_t = konst.tile([128, 128], F32R)
            nc.sync.dma_start(out=smat_t, in_=smat_d[:, :].bitcast(F32R))

            for _rep in range(repeat):
                qkv = tc.alloc_tile_pool(name=f"qkv{_rep}", bufs=1)
                # ---- collective DRAM tiles ----
                agkv_in = dram.tile([LAT, R], BF16, name=f"agkvi{_rep}")
                agkv_out = dram.tile([NC, LAT, R], BF16,
                                     addr_space=("Local" if sim_mode else "Shared"),
                                     name=f"agkvo{_rep}")
                agq_in = dram.tile([QLR, R], BF16, name=f"agqi{_rep}")
                agq_out = dram.tile([NC, QLR, R], BF16,
                                    addr_space=("Local" if sim_mode else "Shared"),
                                    name=f"agqo{_rep}")
                a2a_in = dram.tile([NC, HPC * VD, R], BF16, name=f"a2a_in{_rep}")
                a2a_out = dram.tile([NC, HPC * VD, R], BF16, name=f"a2a_out{_rep}")
                # ---- persistent packed q/k (fp8, nope+rope planes) & v ----
                # plane 0: nope dims; plane 1 partitions h*64:(h+1)*64: rope
                qpk = [qkv.tile([128, 2, N], F8, name=f"qpk{i}") for i in range(HPC)]
                kpk = [qkv.tile([128, 2, N], F8, name=f"kpk{i}") for i in range(HPC)]
                vt = qkv.tile([128, N // 128, HPC * VD], BF16)   # token-major V

                # zero the unused rope half-planes (head h uses rows h*64:h*64+64)
                nc.vector.memset(qpk[0][64:128, 1, :], 0.0)
                nc.vector.memset(qpk[1][0:64, 1, :], 0.0)
                nc.vector.memset(kpk[0][64:128, 1, :], 0.0)
                nc.vector.memset(kpk[1][0:64, 1, :], 0.0)

                # ================= PHASE 1: latent down-proj (row shard) ============
                # kv path first so its AllGather overlaps the q path compute.
                with tc.tile_pool(name=f"p1sb{_rep}", bufs=1) as p1sb, \
                     tc.tile_pool(name=f"p1w{_rep}", bufs=3) as p1w, \
                     tc.tile_pool(name=f"p1tmp{_rep}", bufs=2) as p1tmp, \
                     tc.tile_pool(name=f"p1ps{_rep}", bufs=3, space="PSUM") as p1ps, \
                     tc.tile_pool(name=f"p1ps2{_rep}", bufs=2, space="PSUM") as p1ps2, \
                     tc.tile_pool(name=f"p1ps1{_rep}", bufs=1, space="PSUM") as p1ps1:

                    t1c = p1sb.tile([64, R], BF16)
                    nc.sync.dma_start(out=t1c, in_=t1c_d[:, :])
                    t2c = p1sb.tile([64, R], BF16)
                    nc.sync.dma_start(out=t2c, in_=t2c_d[:, :])

                    xts = p1sb.tile([128, DIM // 128, R], BF16)
                    nc.sync.dma_start(out=xts, in_=x_c[:, :, :])

                    for path in (1, 0):  # 1: kv (first), 0: q
                        wT, bias_d, normw_d = ((wqaT, bqa, qnw) if path == 0
                                               else (wkvaT, bkva, kvnw))
                        ag_dst = agq_in if path == 0 else agkv_in
                        nm = 4 if path == 0 else 5  # kv has extra 64-row rope chunk
                        acts = []
                        sums_ps = p1ps1.tile([1, R], F32, name=f"sums{path}",
                                             tag="sums")
                        for m in range(nm):
                            wt = p1w.tile([128, DIM // 128, 128], BF16, tag="w1")
                            nc.sync.dma_start(out=wt, in_=wT[m, :, :, :])
                            ps = p1ps.tile([128, R], F32, tag="p1acc")
                            for k in range(DIM // 128):
                                nc.tensor.matmul(ps[:, :], wt[:, k, :], xts[:, k, :],
                                                 start=(k == 0), stop=(k == DIM // 128 - 1))
                            if m < 4:   # latent chunks
                                a = p1sb.tile([128, R], F32, tag=f"act{m}",
                                              name=f"a{path}{m}")
                                if flags['ba'][path]:
                                    bt = p1sb.tile([128, 1], F32, tag=f"bias{m}",
                                                   name=f"b{path}{m}")
                                    nc.sync.dma_start(out=bt,
                                                      in_=bias_d[m * 128:(m + 1) * 128]
                                                      .rearrange("(a b) -> a b", b=1))
                                    nc.vector.tensor_scalar_add(a[:, :], ps[:, :],
                                                                bt[:, :])
                                else:
                                    nc.scalar.activation(out=a[:, :], in_=ps[:, :],
                                                         func=AF.Copy)
                                acts.append(a)
                                sq = p1tmp.tile([128, R], F32R, tag="sq")
                                nc.vector.tensor_mul(sq[:, :], a[:, :], a[:, :])
                                nc.tensor.matmul(sums_ps[:, :], ones_col[:, :], sq[:, :],
                                                 start=(m == 0), stop=(m == 3),
                                                 skip_group_check=True)
                            else:       # kv rope chunk [64, R]
                                if flags['ba'][path]:
                                    bt = p1sb.tile([64, 1], F32, tag="bias4",
                                                   name="bkpe")
                                    nc.sync.dma_start(out=bt,
                                                      in_=bias_d[512:576]
                                                      .rearrange("(a b) -> a b", b=1))
                                    t1 = p1tmp.tile([64, R], F32R, tag="t1")
                                    nc.vector.scalar_tensor_tensor(
                                        out=t1[:, :], in0=ps[0:64, :], scalar=bt[:, :],
                                        in1=t1c[:, :], op0=ALU.add, op1=ALU.mult)
                                    t2 = p1tmp.tile([64, R], F32R, tag="t2")
                                    nc.vector.scalar_tensor_tensor(
                                        out=t2[:, :], in0=ps[0:64, :], scalar=bt[:, :],
                                        in1=t2c[:, :], op0=ALU.add, op1=ALU.mult)
                                else:
                                    t1 = p1tmp.tile([64, R], F32R, tag="t1")
                                    nc.vector.tensor_mul(t1[:, :], ps[0:64, :], t1c[:, :])
                                    t2 = p1tmp.tile([64, R], F32R, tag="t2")
                                    nc.vector.tensor_mul(t2[:, :], ps[0:64, :], t2c[:, :])
                                psr = p1ps2.tile([64, R], F32, tag="psr")
                                nc.tensor.matmul(psr[:, :], smat_t[0:64, 0:64],
                                                 t2[:, :], start=True, stop=True)
                                rot = p1tmp.tile([64, R], BF16, tag="rot")
                                nc.vector.tensor_add(rot[:, :], psr[:, :], t1[:, :])
                                nc.sync.dma_start(out=agkv_in[KVLR:KVLR + ROPE, :],
                                                  in_=rot[:, :])
                        # rstd = 1/sqrt(mean + eps), fold norm weight via matmul
                        std = p1tmp.tile([1, R], F32, tag="std")
                        nc.scalar.activation(out=std[:, :], in_=sums_ps[:, :],
                                             func=AF.Sqrt,
                                             scale=1.0 / (QLR if path == 0 else KVLR),
                                             bias=eps_t[:, :])
                        rstd_f = p1tmp.tile([1, R], F32, tag="rstdf")
                        nc.vector.reciprocal(out=rstd_f[:, :], in_=std[:, :])
                        rstd = p1tmp.tile([1, R], F32R, tag="rstd")
                        nc.vector.tensor_copy(out=rstd[:, :], in_=rstd_f[:, :])
                        for m in range(4):
                            wrow = p1sb.tile([1, 128], F32R, tag=f"wrow{m}",
                                             name=f"w{path}{m}")
                            nc.sync.dma_start(out=wrow,
                                              in_=normw_d[m * 128:(m + 1) * 128]
                                              .rearrange("(b a) -> b a", b=1).bitcast(F32R))
                            rep = p1ps2.tile([128, R], F32, tag="p1rep")
                            nc.tensor.matmul(rep[:, :], wrow[:, :], rstd[:, :],
                                             start=True, stop=True)
                            nrm = p1tmp.tile([128, R], BF16, tag="nrm")
                            nc.vector.tensor_mul(nrm[:, :], acts[m][:, :], rep[:, :])
                            nc.sync.dma_start(
                                out=ag_dst[m * 128:(m + 1) * 128, :],
                                in_=nrm[:, :])
                        acts.clear()
                        # kick this path's AllGather as soon as its rows land
                        ag_i, ag_o = ((agq_in, agq_out) if path == 0
                                      else (agkv_in, agkv_out))
                        if sim_mode:
                            nc.sync.dma_start(out=ag_o[0, 0:1, :], in_=ag_i[0:1, :])
                        else:
                            nc.gpsimd.collective_compute(
                                "AllGather", ALU.bypass,
                                replica_groups=[list(range(NC))],
                                ins=[ag_i.opt()], outs=[ag_o.opt()])

                # ================= PHASE 2: per-head up-projections ================
                with tc.tile_pool(name=f"p2w{_rep}", bufs=1) as p2w, \
                     tc.tile_pool(name=f"p2lat{_rep}", bufs=2) as p2lat, \
                     tc.tile_pool(name=f"p2tmp{_rep}", bufs=2) as p2tmp, \
                     tc.tile_pool(name=f"p2ps{_rep}", bufs=3, space="PSUM") as p2ps, \
                     tc.tile_pool(name=f"p2psr{_rep}", bufs=1, space="PSUM") as p2psr:

                    wkb_t = p2w.tile([128, 2, 4, 128], BF16)
                    nc.sync.dma_start(out=wkb_t, in_=wkbT[:, :, :, :])
                    wvb_t = p2w.tile([128, 4, HPC * VD], BF16)
                    nc.sync.dma_start(out=wvb_t, in_=wvbT[:, :, :])
                    if flags['bvb']:
                        bvb_t = p2w.tile([1, HPC * VD], F32R)
                        nc.sync.dma_start(out=bvb_t, in_=bvb[:, :].bitcast(F32R))
                    if flags['bkb']:
                        bk_t = [p2w.tile([128, 1], F32, name=f"bkt{m}") for m in range(2)]
                        for m in range(2):
                            nc.sync.dma_start(out=bk_t[m],
                                              in_=bkb[m * 128:(m + 1) * 128]
                                              .rearrange("(a b) -> a b", b=1))

                    # ---- pass A: k_nope / k_pe / v from the kv AllGather ----
                    for s in range(NBLK):
                        tsl = slice(s * R, (s + 1) * R)
                        kn = p2lat.tile([128, 4, R], BF16, tag="kn")
                        nc.sync.dma_start(out=kn,
                                          in_=agkv_out[s, 0:KVLR, :]
                                          .rearrange("(k p) t -> p k t", p=128))
                        kpe2 = p2lat.tile([128, R], BF16, tag="kpe")
                        nc.sync.dma_start(out=kpe2[0:64, :],
                                          in_=agkv_out[s, KVLR:KVLR + ROPE, :])
                        nc.sync.dma_start(out=kpe2[64:128, :],
                                          in_=agkv_out[s, KVLR:KVLR + ROPE, :])
                        for m in range(2):   # head m k_nope
                            ps = p2ps.tile([128, R], F32, tag="p2acc")
                            for k in range(4):
                                nc.tensor.matmul(ps[:, :], wkb_t[:, m, k, :],
                                                 kn[:, k, :],
                                                 start=(k == 0), stop=(k == 3))
                            if flags['bkb']:
                                nc.vector.tensor_scalar_add(kpk[m][:, 0, tsl],
                                                            ps[:, :], bk_t[m][:, :])
                            else:
                                nc.scalar.activation(out=kpk[m][:, 0, tsl],
                                                     in_=ps[:, :], func=AF.Copy)
                        nc.vector.tensor_copy(out=kpk[0][0:64, 1, tsl],
                                              in_=kpe2[0:64, :])
                        nc.vector.tensor_copy(out=kpk[1][64:128, 1, tsl],
                                              in_=kpe2[64:128, :])
                        for mt in range(4):  # v (token-major)
                            ps = p2ps.tile([128, HPC * VD], F32, tag="p2v")
                            if flags['bvb']:
                                nc.tensor.matmul(ps[:, :], ones_row[:, :], bvb_t[:, :],
                                                 start=True, stop=False)
                            for k in range(4):
                                nc.tensor.matmul(
                                    ps[:, :],
                                    kn[:, k, mt * 128:(mt + 1) * 128],
                                    wvb_t[:, k, :],
                                    start=(k == 0 and not flags['bvb']),
                                    stop=(k == 3))
                            nc.scalar.activation(out=vt[:, s * 4 + mt, :], in_=ps[:, :],
                                                 func=AF.Copy)

                    # ---- pass B: q_nope / q_rope from the q AllGather ----
                    tq1 = p2w.tile([128, S], BF16)
                    nc.sync.dma_start(out=tq1, in_=t1q_d[:, :])
                    tq2 = p2w.tile([128, S], BF16)
                    nc.sync.dma_start(out=tq2, in_=t2q_d[:, :])
                    wqb_t = p2w.tile([128, 3, 4, 128], BF16)
                    nc.sync.dma_start(out=wqb_t, in_=wqbT[:, :, :, :])
                    if flags['bqb']:
                        bq_t = [p2w.tile([128, 1], F32, name=f"bqt{m}") for m in range(3)]
                        for m in range(3):
                            nc.sync.dma_start(out=bq_t[m],
                                              in_=bqb[m * 128:(m + 1) * 128]
                                              .rearrange("(a b) -> a b", b=1))

                    for s in range(NBLK):
                        tsl = slice(s * R, (s + 1) * R)
                        pos = (s % (S // R)) * R       # position within batch
                        psl = slice(pos, pos + R)
                        qn = p2lat.tile([128, 4, R], BF16, tag="qn")
                        nc.sync.dma_start(out=qn,
                                          in_=agq_out[s, :, :]
                                          .rearrange("(k p) t -> p k t", p=128))
                        for m in range(3):
                            ps = p2ps.tile([128, R], F32, tag="p2acc")
                            for k in range(4):
                                nc.tensor.matmul(ps[:, :], wqb_t[:, m, k, :],
                                                 qn[:, k, :],
                                                 start=(k == 0), stop=(k == 3))
                            if m < 2:    # head m q_nope
                                if flags['bqb']:
                                    nc.vector.tensor_scalar_add(qpk[m][:, 0, tsl],
                                                                ps[:, :], bq_t[m][:, :])
                                else:
                                    nc.scalar.activation(out=qpk[m][:, 0, tsl],
                                                         in_=ps[:, :], func=AF.Copy)
                            else:        # rope rows for both heads, split layout
                                if flags['bqb']:
                                    t1 = p2tmp.tile([128, R], F32R, tag="t1")
                                    nc.vector.scalar_tensor_tensor(
                                        out=t1[:, :], in0=ps[:, :],
                                        scalar=bq_t[2][:, :], in1=tq1[:, psl],
                                        op0=ALU.add, op1=ALU.mult)
                                    t2 = p2tmp.tile([128, R], F32R, tag="t2")
                                    nc.vector.scalar_tensor_tensor(
                                        out=t2[:, :], in0=ps[:, :],
                                        scalar=bq_t[2][:, :], in1=tq2[:, psl],
                                        op0=ALU.add, op1=ALU.mult)
                                else:
                                    t1 = p2tmp.tile([128, R], F32R, tag="t1")
                                    nc.vector.tensor_mul(t1[:, :], ps[:, :], tq1[:, psl])
                                    t2 = p2tmp.tile([128, R], F32R, tag="t2")
                                    nc.vector.tensor_mul(t2[:, :], ps[:, :], tq2[:, psl])
                                psr = p2psr.tile([128, R], F32, tag="psr")
                                nc.tensor.matmul(psr[:, :], smat_t[:, :], t2[:, :],
                                                 start=True, stop=True)
                                nc.vector.tensor_add(qpk[0][0:64, 1, tsl],
                                                     psr[0:64, :], t1[0:64, :])
                                nc.vector.tensor_add(qpk[1][64:128, 1, tsl],
                                                     psr[64:128, :], t1[64:128, :])

                # ================= PHASE 3: attention =============================
                with tc.tile_pool(name=f"p3m{_rep}", bufs=1) as p3m, \
                     tc.tile_pool(name=f"p3p{_rep}", bufs=3) as p3p, \
                     tc.tile_pool(name=f"p3o{_rep}", bufs=3) as p3o, \
                     tc.tile_pool(name=f"p3sc{_rep}", bufs=3, space="PSUM") as p3sc, \
                     tc.tile_pool(name=f"p3out{_rep}", bufs=2, space="PSUM") as p3out, \
                     tc.tile_pool(name=f"p3rs{_rep}", bufs=2, space="PSUM") as p3rs, \
                     tc.tile_pool(name=f"p3rep{_rep}", bufs=1, space="PSUM") as p3rep:

                    mtiles = [p3m.tile([128, 512], BF16, name=f"mt{i}")
                              for i in range(nmask)]
                    for i in range(nmask):
                        nc.sync.dma_start(out=mtiles[i], in_=maskblk[i, :, :])

                    for b in range(B):
                        for lh in range(HPC):
                            for qc in range(S // 512):
                                qsl = slice(b * S + qc * 512, b * S + (qc + 1) * 512)
                                out_ps = p3out.tile([128, 512], F32, tag="outp")
                                rs_ps = p3rs.tile([1, 512], F32, tag="rsp")
                                kbs = [kb for kb in range(S // 128)
                                       if cls[qc][kb] != SKIP]
                                for i, kb in enumerate(kbs):
                                    ksl = slice(b * S + kb * 128, b * S + kb * 128 + 128)
                                    sc = p3sc.tile([128, 512], F32, tag="sc")
                                    nc.tensor.matmul(sc[:, :], kpk[lh][:, :, ksl],
                                                     qpk[lh][:, :, qsl],
                                                     start=True, stop=True,
                                                     perf_mode=DR)
                                    if cls[qc][kb] >= 0:
                                        nc.vector.tensor_add(sc[:, :], sc[:, :],
                                                             mtiles[cls[qc][kb]][:, :])
                                    P = p3p.tile([128, 512], BF16, tag="P")
                                    nc.scalar.activation(out=P[:, :], in_=sc[:, :],
                                                         func=AF.Exp, scale=SCALE)
                                    last = (i == len(kbs) - 1)
                                    nc.tensor.matmul(
                                        out_ps[:, :],
                                        vt[:, b * 16 + kb, lh * VD:(lh + 1) * VD],
                                        P[:, :], start=(i == 0), stop=last,
                                        skip_group_check=True)
                                    nc.tensor.matmul(rs_ps[:, :], ones_bcol[:, :],
                                                     P[:, :], start=(i == 0), stop=last,
                                                     skip_group_check=True)
                                inv_f = p3o.tile([1, 512], F32, tag="invf")
                                nc.vector.reciprocal(out=inv_f[:, :], in_=rs_ps[:, :])
                                inv = p3o.tile([1, 512], F32R, tag="inv")
                                nc.vector.tensor_copy(out=inv[:, :], in_=inv_f[:, :])
                                rep = p3rep.tile([128, 512], F32, tag="rep")
                                nc.tensor.matmul(rep[:, :], ones_row[:, :], inv[:, :],
                                                 start=True, stop=True)
                                rep_sb = p3o.tile([128, 512], F32, tag="repsb")
                                nc.scalar.activation(out=rep_sb[:, :], in_=rep[:, :],
                                                     func=AF.Copy)
                                ao = p3o.tile([128, 512], BF16, tag="ao")
                                nc.vector.tensor_mul(ao[:, :], out_ps[:, :], rep_sb[:, :])
                                nc.sync.dma_start(
                                    out=a2a_in[b * 4 + qc, lh * VD:(lh + 1) * VD, :],
                                    in_=ao[:, :])

                qkv.release()

                # ---- AllToAll ----
                if sim_mode:
                    nc.sync.dma_start(out=a2a_out[0, 0:1, :], in_=a2a_in[0, 0:1, :])
                else:
                    nc.gpsimd.collective_compute(
                        "AllToAll", ALU.bypass,
                        replica_groups=[list(range(NC))],
                        ins=[a2a_in.opt()], outs=[a2a_out.opt()])

                # ================= PHASE 4: output projection =====================
                with tc.tile_pool(name=f"p4l{_rep}", bufs=1) as p4l, \
                     tc.tile_pool(name=f"p4r{_rep}", bufs=2) as p4r, \
                     tc.tile_pool(name=f"p4o{_rep}", bufs=3) as p4o, \
                     tc.tile_pool(name=f"p4ps{_rep}", bufs=4, space="PSUM") as p4ps:

                    lt = p4l.tile([128, 16, 512], BF16)
                    av = a2a_out[:, :, :].rearrange("c (h p) t -> (c h) p t", h=2)
                    for k in range(16):
                        nc.sync.dma_start(out=lt[:, k, :], in_=av[k, :, :])
                    if flags['wob']:
                        wob_t = p4l.tile([1, DIM], F32R)
                        nc.sync.dma_start(out=wob_t, in_=wob[:, :].bitcast(F32R))
                    for n_ in range(4):
                        rh = p4r.tile([128, 16, 512], BF16, tag="rh")
                        nc.sync.dma_start(out=rh, in_=woT[n_, :, :, :])
                        for m in range(4):
                            ps = p4ps.tile([128, 512], F32, tag="p4acc")
                            if flags['wob']:
                                nc.tensor.matmul(ps[:, :], ones_row[:, :],
                                                 wob_t[:, n_ * 512:(n_ + 1) * 512],
                                                 start=True, stop=False)
                            for k in range(16):
                                nc.tensor.matmul(ps[:, :],
                                                 lt[:, k, m * 128:(m + 1) * 128],
                                                 rh[:, k, :],
                                                 start=(k == 0 and not flags['wob']),
                                                 stop=(k == 15))
                            ob = p4o.tile([128, 512], F32, tag="ob")
                            nc.scalar.activation(out=ob[:, :], in_=ps[:, :], func=AF.Copy)
                            nc.sync.dma_start(
                                out=out_c[m * 128:(m + 1) * 128,
                                          n_ * 512:(n_ + 1) * 512],
                                in_=ob[:, :])

    nc.finalize()
    return nc


_ROPE_PERM = np.concatenate([np.arange(0, ROPE, 2), np.arange(1, ROPE, 2)])

_CACHE = {}


def _prep_inputs(inputs):
    """Host-side slicing/permutation -> (schedule key data, per-core in_maps)."""
    x = np.ascontiguousarray(np.asarray(inputs['x'], np.float32).reshape(N, DIM))
    mask = np.asarray(inputs['mask'])
    cls, blocks = _classify_mask(mask)

    cos_t, sin_t = _rope_tables()            # [32, S]
    t1q = np.concatenate([cos_t, cos_t, cos_t, cos_t], 0).astype(BF)   # [128, S]
    t2q = np.concatenate([sin_t, -sin_t, sin_t, -sin_t], 0).astype(BF)
    s64 = np.zeros((64, 64), np.float32)
    s64[0:32, 32:64] = np.eye(32)
    s64[32:64, 0:32] = np.eye(32)
    smat = np.zeros((128, 128), np.float32)
    smat[0:64, 0:64] = s64
    smat[64:128, 64:128] = s64

    wq_a = np.asarray(inputs['wq_a_w'], np.float32)            # [QLR, DIM]
    wkv_a = np.asarray(inputs['wkv_a_w'], np.float32)          # [KVLR+ROPE, DIM]
    wkv_a_p = np.concatenate([wkv_a[:KVLR], wkv_a[KVLR:][_ROPE_PERM]], 0)
    bkva = np.asarray(inputs['wkv_a_b'], np.float32)
    bkva_p = np.concatenate([bkva[:KVLR], bkva[KVLR:][_ROPE_PERM]], 0)

    wq_b = np.asarray(inputs['wq_b_w'], np.float32).reshape(H, QKD, QLR)
    bq_b = np.asarray(inputs['wq_b_b'], np.float32).reshape(H, QKD)
    wkv_b = np.asarray(inputs['wkv_b_w'], np.float32).reshape(H, NOPE + VD, KVLR)
    bkv_b = np.asarray(inputs['wkv_b_b'], np.float32).reshape(H, NOPE + VD)
    wo = np.asarray(inputs['wo_w'], np.float32)                # [DIM, H*VD]

    def blk1(w2d, nm):
        # [out, DIM] -> [nm, 128(part=x chunk), 16(k), 128(out cols)]
        t = np.ascontiguousarray(w2d.T)                        # [DIM, out]
        return np.ascontiguousarray(
            t.reshape(DIM // 128, 128, nm, 128).transpose(2, 1, 0, 3)).astype(BF)

    def blk2(w2d, nm):
        # [out(nm*128), 512] -> [128(part=latent chunk), nm, 4(k), 128]
        t = np.ascontiguousarray(w2d.T)                        # [512, out]
        return np.ascontiguousarray(
            t.reshape(4, 128, nm, 128).transpose(1, 2, 0, 3)).astype(BF)

    wkva_pad = np.concatenate([wkv_a_p, np.zeros((64, DIM), np.float32)], 0)
    woT = np.ascontiguousarray(
        wo.T.reshape(16, 128, 4, 512).transpose(2, 1, 0, 3)).astype(BF)

    shared = {
        'wqaT': blk1(wq_a, 4),
        'bqa': np.asarray(inputs['wq_a_b'], np.float32),
        'wkvaT': blk1(wkva_pad, 5),
        'bkva': bkva_p,
        'qnw': np.asarray(inputs['q_norm_w'], np.float32),
        'kvnw': np.asarray(inputs['kv_norm_w'], np.float32),
        't1q': t1q,
        't2q': t2q,
        'smat': smat,
        'woT': woT,
        'wob': np.asarray(inputs['wo_b'], np.float32)[None, :],
        'maskblk': blocks,
    }

    in_maps = []
    for c in range(NC):
        h0, h1 = 2 * c, 2 * c + 1
        # q_b rows: h0 nope, h1 nope, [h0 rope-e, h0 rope-o, h1 rope-e, h1 rope-o]
        wqb_c = np.concatenate([
            wq_b[h0, :NOPE], wq_b[h1, :NOPE],
            wq_b[h0, NOPE:][_ROPE_PERM], wq_b[h1, NOPE:][_ROPE_PERM]], 0)
        bqb_c = np.concatenate([
            bq_b[h0, :NOPE], bq_b[h1, :NOPE],
            bq_b[h0, NOPE:][_ROPE_PERM], bq_b[h1, NOPE:][_ROPE_PERM]], 0)
        wkb_c = np.concatenate([wkv_b[h0, :NOPE], wkv_b[h1, :NOPE]], 0)
        bkb_c = np.concatenate([bkv_b[h0, :NOPE], bkv_b[h1, :NOPE]], 0)
        wvb_c = np.concatenate([wkv_b[h0, NOPE:], wkv_b[h1, NOPE:]], 0)
        bvb_c = np.concatenate([bkv_b[h0, NOPE:], bkv_b[h1, NOPE:]], 0)
        pos = (c % (S // R)) * R
        cos_c, sin_c = cos_t[:, pos:pos + R], sin_t[:, pos:pos + R]
        m = dict(shared)
        m.update({
            'x_c': np.ascontiguousarray(
                x[c * R:(c + 1) * R].T.reshape(DIM // 128, 128, R)
                .transpose(1, 0, 2)).astype(BF),
            'wqbT': blk2(wqb_c, 3),
            'bqb': bqb_c,
            'wkbT': blk2(wkb_c, 2),
            'bkb': bkb_c,
            'wvbT': np.ascontiguousarray(
                wvb_c.T.reshape(4, 128, HPC * VD).transpose(1, 0, 2)).astype(BF),
            'bvb': bvb_c[None, :],
            't1c': np.ascontiguousarray(
                np.concatenate([cos_c, cos_c], 0)).astype(BF),
            't2c': np.ascontiguousarray(
                np.concatenate([sin_c, -sin_c], 0)).astype(BF),
        })
        in_maps.append(m)
    return cls, in_maps


class _Runner:
    """Compile once, execute many times on the 8 axon-tunneled NeuronCores."""

    def __init__(self, nc):
        import jax
        from jax.experimental.shard_map import shard_map
        from jax.sharding import Mesh, PartitionSpec
        from concourse import bass2jax, mybir as _mybir
        bass2jax.install_neuronx_cc_hook()
        self.jax = jax
        in_names, out_names, out_avals, zero_outs = [], [], [], []
        partition_name = (nc.partition_id_tensor.name
                          if nc.partition_id_tensor else None)
        for alloc in nc.m.functions[0].allocations:
            if not isinstance(alloc, _mybir.MemoryLocationSet):
                continue
            name = alloc.memorylocations[0].name
            if alloc.kind == "ExternalInput":
                if name != partition_name:
                    in_names.append(name)
            elif alloc.kind == "ExternalOutput":
                shape = tuple(alloc.tensor_shape)
                dtype = _mybir.dt.np(alloc.dtype)
                out_names.append(name)
                out_avals.append(jax.core.ShapedArray(shape, dtype))
                zero_outs.append(np.zeros(shape, dtype))
        self.n_params = len(in_names)
        self.in_names = list(in_names)
        self.out_names = out_names
        self.out_avals = out_avals
        self.zero_outs = zero_outs
        all_in = in_names + out_names
        if partition_name is not None:
            all_in.append(partition_name)

        def _body(*args):
            operands = list(args)
            if partition_name is not None:
                operands.append(bass2jax.partition_id_tensor())
            outs = bass2jax._bass_exec_p.bind(
                *operands,
                out_avals=tuple(out_avals),
                in_names=tuple(all_in),
                out_names=tuple(out_names),
                lowering_input_output_aliases=(),
                sim_require_finite=True,
                sim_require_nnan=True,
                nc=nc)
            return tuple(outs)

        devices = jax.devices()[:NC]
        self.mesh = Mesh(np.asarray(devices), ("core",))
        n_out = len(out_names)
        in_specs = (PartitionSpec("core"),) * (self.n_params + n_out)
        out_specs = (PartitionSpec("core"),) * n_out
        donate = tuple(range(self.n_params, self.n_params + n_out))
        self.fn = jax.jit(
            shard_map(_body, mesh=self.mesh, in_specs=in_specs,
                      out_specs=out_specs, check_rep=False),
            donate_argnums=donate, keep_unused=True)

    def concat_inputs(self, in_maps):
        return [np.concatenate([np.asarray(in_maps[c][nm])
                                for c in range(NC)], axis=0)
                for nm in self.in_names]

    def zeros(self):
        return [np.zeros((NC * z.shape[0], *z.shape[1:]), z.dtype)
                for z in self.zero_outs]

    def __call__(self, concat_in, concat_zeros):
        out = self.fn(*concat_in, *concat_zeros)
        return out

    def run(self, in_maps):
        outs = self(self.concat_inputs(in_maps), self.zeros())
        res = []
        for c in range(NC):
            res.append({nm: np.asarray(outs[i]).reshape(NC, *self.out_avals[i].shape)[c]
                        for i, nm in enumerate(self.out_names)})
        return res


def _get_exec(cls, nmask, flags):
    key = (tuple(tuple(r) for r in cls), nmask,
           tuple(flags['ba']), flags['bqb'], flags['bkb'], flags['bvb'],
           flags['wob'])
    if key not in _CACHE:
        nc = _build(cls, nmask, flags)
        _CACHE[key] = _Runner(nc)
    return _CACHE[key]


def kernel(**inputs):
    cls, in_maps = _prep_inputs(inputs)
    nmask = max(len(in_maps[0]['maskblk']), 1)
    flags = {
        'ba': (bool(np.any(inputs['wq_a_b'])), bool(np.any(inputs['wkv_a_b']))),
        'bqb': bool(np.any(inputs['wq_b_b'])),
        'bkb': bool(np.any(np.asarray(inputs['wkv_b_b']).reshape(H, NOPE + VD)[:, :NOPE])),
        'bvb': bool(np.any(np.asarray(inputs['wkv_b_b']).reshape(H, NOPE + VD)[:, NOPE:])),
        'wob': bool(np.any(inputs['wo_b'])),
    }
    runner = _get_exec(cls, nmask, flags)
    results = runner.run(in_maps)
    out = np.concatenate([results[c]["out"] for c in range(NC)], 0)
    return out.reshape(B, S, DIM)


# revision 47
# speedup vs baseline: 10.2627x; 10.2627x over previous
"""Multi-Head Latent Attention (MLA) prefill kernel for 8 Trainium2 NeuronCores.

Sharding: latent down-projections row-split 8 ways + AllGather (split kv/q for
overlap); up-projections and attention head-split (2 heads/core); AllToAll
converts head-split attention output to token-split for the output projection.

Precision: bf16 weights/activations end-to-end (measured 4.7e-3 rel err),
fp8e4 DoubleRow matmuls for attention scores (q/k packed nope+rope in a
256-deep contraction, 4x fewer PE cycles; measured 1.15e-2 rel err total);
P and V stay bf16 (fp8 there fails the 2e-2 budget). f32 PSUM throughout.
"""
import sys
if '/opt/trn_rl_repo' not in sys.path:
    sys.path.insert(0, '/opt/trn_rl_repo')

import math
import numpy as np
import ml_dtypes

import concourse.bass as bass
import concourse.tile as tile
import concourse.mybir as mybir
from concourse import bacc

F32 = mybir.dt.float32
F32R = mybir.dt.float32r
BF16 = mybir.dt.bfloat16
F8 = mybir.dt.float8e4
AF = mybir.ActivationFunctionType
ALU = mybir.AluOpType
DR = mybir.MatmulPerfMode.DoubleRow
BF = ml_dtypes.bfloat16

B, S, DIM, H = 2, 2048, 2048, 16
NOPE, ROPE, QKD, VD = 128, 64, 192, 128
QLR, KVLR = 512, 512
EPS = 1e-6
NC = 8
N = B * S              # 4096 flattened tokens
R = N // NC            # 512 tokens per core (phase 1 / output rows)
HPC = H // NC          # 2 heads per core
NBLK = N // R          # 8 token blocks (= AG shards)
SCALE = 1.0 / math.sqrt(QKD)
NEG = -240.0           # additive mask value, fp8e4-exact; exp(NEG*SCALE) ~ 3e-8

SKIP, PLAIN = -2, -1   # mask block classes (>=0 -> index into mask blocks)


def _rope_tables():
    freqs = (1.0 / (10000.0 ** (np.arange(0, ROPE, 2, dtype=np.float32) / ROPE)))
    ang = np.arange(S, dtype=np.float32)[:, None] * freqs[None, :]      # [S, 32]
    return np.cos(ang).T.copy(), np.sin(ang).T.copy()                   # [32, S]


def _classify_mask(mask):
    """Per (q-chunk of 512, k-block of 128): SKIP / PLAIN / index into deduped
    additive mask blocks (0 where allowed, NEG where masked), [128 k, 512 q]."""
    cls = [[PLAIN] * (S // 128) for _ in range(S // 512)]
    blocks, keys = [], {}
    for qc in range(S // 512):
        sub_q = mask[qc * 512:(qc + 1) * 512]
        for kb in range(S // 128):
            blk = np.asarray(sub_q[:, kb * 128:(kb + 1) * 128])
            if not blk.any():
                cls[qc][kb] = SKIP
            elif blk.all():
                cls[qc][kb] = PLAIN
            else:
                key = blk.tobytes()
                if key not in keys:
                    keys[key] = len(blocks)
                    m8 = np.zeros((128, 2, 512), ml_dtypes.float8_e4m3fn)
                    m8[:, 0, :] = np.where(blk.T, 0.0, NEG)
                    blocks.append(m8.view(np.uint8))
                cls[qc][kb] = keys[key]
    blocks = (np.stack(blocks) if blocks
              else np.zeros((1, 128, 2, 512), np.uint8))
    return cls, blocks


def _build(cls, nmask, flags, repeat=1, sim_mode=False):
    """Emit the bass program. cls/nmask/flags are compile-time schedule data."""
    nc = bacc.Bacc(None, num_devices=NC)

    LAT = KVLR + ROPE  # 576

    # ---- I/O ----
    x_c = nc.dram_tensor("x_c", [128, DIM // 128, R], BF16, kind="ExternalInput")
    wqaT = nc.dram_tensor("wqaT", [4, 128, DIM // 128, 128], BF16, kind="ExternalInput")
    wkvaT = nc.dram_tensor("wkvaT", [5, 128, DIM // 128, 128], BF16, kind="ExternalInput")
    bqa = nc.dram_tensor("bqa", [QLR], F32, kind="ExternalInput")
    bkva = nc.dram_tensor("bkva", [KVLR + ROPE], F32, kind="ExternalInput")
    qnw = nc.dram_tensor("qnw", [QLR], F32, kind="ExternalInput")
    kvnw = nc.dram_tensor("kvnw", [KVLR], F32, kind="ExternalInput")
    # trig tables for split-halves rope: t1 = [c;c], t2 = [s;-s]
    t1c_d = nc.dram_tensor("t1c", [64, R], BF16, kind="ExternalInput")
    t2c_d = nc.dram_tensor("t2c", [64, R], BF16, kind="ExternalInput")
    t1q_d = nc.dram_tensor("t1q", [128, S], BF16, kind="ExternalInput")
    t2q_d = nc.dram_tensor("t2q", [128, S], BF16, kind="ExternalInput")
    smat_d = nc.dram_tensor("smat", [128, 128], F32, kind="ExternalInput")
    wqbT = nc.dram_tensor("wqbT", [128, 3, 4, 128], BF16, kind="ExternalInput")
    bqb = nc.dram_tensor("bqb", [HPC * QKD], F32, kind="ExternalInput")
    wkbT = nc.dram_tensor("wkbT", [128, 2, 4, 128], BF16, kind="ExternalInput")
    bkb = nc.dram_tensor("bkb", [HPC * NOPE], F32, kind="ExternalInput")
    wvbT = nc.dram_tensor("wvbT", [128, 4, HPC * VD], BF16, kind="ExternalInput")
    bvb = nc.dram_tensor("bvb", [1, HPC * VD], F32, kind="ExternalInput")
    woT = nc.dram_tensor("woT", [4, 128, 16, 512], BF16, kind="ExternalInput")
    wob = nc.dram_tensor("wob", [1, DIM], F32, kind="ExternalInput")
    # fp8 payloads travel as uint8 (fp8 I/O dtypes are rejected at the HLO
    # level on trn2) and are bitcast on load
    maskblk = nc.dram_tensor("maskblk", [max(nmask, 1), 128, 2, 512], mybir.dt.uint8,
                             kind="ExternalInput")
    id8_d = nc.dram_tensor("id8", [128, 2, 128], mybir.dt.uint8, kind="ExternalInput")
    out_c = nc.dram_tensor("out", [R, DIM], F32, kind="ExternalOutput")

    with tile.TileContext(nc) as tc:
        with tc.tile_pool(name="konst", bufs=1) as konst, \
             tc.tile_pool(name="dram", bufs=1, space="DRAM") as dram:

            # ---- constants ----
            ones_f = konst.tile([128, 1], F32)
            nc.vector.memset(ones_f[:, :], 1.0)
            ones_col = konst.tile([128, 1], F32R)
            nc.vector.tensor_copy(out=ones_col[:, :], in_=ones_f[:, :])
            ones_bcol = konst.tile([128, 1], BF16)
            nc.vector.memset(ones_bcol[:, :], 1.0)
            # all-ones stationary: rowsum AND 128-way broadcast in one matmul
            # (narrow DoubleRow ldweights fail the ISA check; full-width passes)
            ones8 = konst.tile([128, 2, 128], F8)
            nc.vector.memset(ones8[:, :, :], 1.0)
            ones_rf = konst.tile([1, 128], F32)
            nc.vector.memset(ones_rf[:, :], 1.0)
            ones_row = konst.tile([1, 128], F32R)
            nc.vector.tensor_copy(out=ones_row[:, :], in_=ones_rf[:, :])
            eps_t = konst.tile([1, 1], F32)
            nc.vector.memset(eps_t[:, :], EPS)
            smat_t = konst.tile([128, 128], F32R)
            nc.sync.dma_start(out=smat_t, in_=smat_d[:, :].bitcast(F32R))
            id8_t = konst.tile([128, 2, 128], F8)
            nc.sync.dma_start(out=id8_t, in_=id8_d[:, :, :].bitcast(F8))

            for _rep in range(repeat):
                p4w = tc.alloc_tile_pool(name=f"p4w{_rep}", bufs=1)
                qkv = tc.alloc_tile_pool(name=f"qkv{_rep}", bufs=1)
                # ---- collective DRAM tiles ----
                agkv_in = dram.tile([LAT, R], BF16, name=f"agkvi{_rep}")
                agkv_out = dram.tile([NC, LAT, R], BF16,
                                     addr_space=("Local" if sim_mode else "Shared"),
                                     name=f"agkvo{_rep}")
                agq_in = dram.tile([QLR, R], BF16, name=f"agqi{_rep}")
                agq_out = dram.tile([NC, QLR, R], BF16,
                                    addr_space=("Local" if sim_mode else "Shared"),
                                    name=f"agqo{_rep}")
                a2a_in = dram.tile([NC, HPC * VD, R], BF16, name=f"a2a_in{_rep}")
                a2a_out = dram.tile([NC, HPC * VD, R], BF16, name=f"a2a_out{_rep}")
                # ---- persistent packed q/k (fp8, nope+rope planes) & v ----
                # plane 0: nope dims; plane 1 partitions h*64:(h+1)*64: rope
                qpk = [qkv.tile([128, 2, N], F8, name=f"qpk{i}") for i in range(HPC)]
                kpk = [qkv.tile([128, 2, N], F8, name=f"kpk{i}") for i in range(HPC)]
                vt = qkv.tile([128, N // 128, HPC * VD], BF16)   # token-major V

                # zero the unused rope half-planes (head h uses rows h*64:h*64+64)
                nc.gpsimd.memset(qpk[0][64:128, 1, :], 0.0)
                nc.gpsimd.memset(qpk[1][0:64, 1, :], 0.0)
                nc.gpsimd.memset(kpk[0][64:128, 1, :], 0.0)
                nc.gpsimd.memset(kpk[1][0:64, 1, :], 0.0)

                # ================= PHASE 1: latent down-proj (row shard) ============
                # kv path first so its AllGather overlaps the q path compute.
                with tc.tile_pool(name=f"p1sb{_rep}", bufs=1) as p1sb, \
                     tc.tile_pool(name=f"p1a{_rep}", bufs=2) as p1a, \
                     tc.tile_pool(name=f"p1w{_rep}", bufs=3) as p1w, \
                     tc.tile_pool(name=f"p1tmp{_rep}", bufs=2) as p1tmp, \
                     tc.tile_pool(name=f"p1ps{_rep}", bufs=3, space="PSUM") as p1ps, \
                     tc.tile_pool(name=f"p1ps2{_rep}", bufs=2, space="PSUM") as p1ps2, \
                     tc.tile_pool(name=f"p1ps1{_rep}", bufs=1, space="PSUM") as p1ps1:

                    t1c = p1sb.tile([64, R], BF16)
                    nc.sync.dma_start(out=t1c, in_=t1c_d[:, :])
                    t2c = p1sb.tile([64, R], BF16)
                    nc.sync.dma_start(out=t2c, in_=t2c_d[:, :])

                    xts = p1sb.tile([128, DIM // 128, R], BF16)
                    for xc in range(4):
                        nc.sync.dma_start(out=xts[:, 4 * xc:4 * xc + 4, :],
                                          in_=x_c[:, 4 * xc:4 * xc + 4, :])

                    for path in (1, 0):  # 1: kv (first), 0: q
                        wT, bias_d, normw_d = ((wqaT, bqa, qnw) if path == 0
                                               else (wkvaT, bkva, kvnw))
                        ag_dst = agq_in if path == 0 else agkv_in
                        nm = 4 if path == 0 else 5  # kv has extra 64-row rope chunk
                        acts = []
                        sums_ps = p1ps1.tile([1, R], F32, name=f"sums{path}",
                                             tag="sums")
                        for m in range(nm):
                            wt = p1w.tile([128, DIM // 128, 128], BF16, tag="w1")
                            nc.sync.dma_start(out=wt, in_=wT[m, :, :, :])
                            ps = p1ps.tile([128, R], F32, tag="p1acc")
                            for k in range(DIM // 128):
                                nc.tensor.matmul(ps[:, :], wt[:, k, :], xts[:, k, :],
                                                 start=(k == 0), stop=(k == DIM // 128 - 1))
                            if m < 4:   # latent chunks
                                a = p1a.tile([128, R], F32, tag=f"act{m}",
                                             name=f"a{path}{m}")
                                if flags['ba'][path]:
                                    bt = p1sb.tile([128, 1], F32, tag=f"bias{m}",
                                                   name=f"b{path}{m}")
                                    nc.sync.dma_start(out=bt,
                                                      in_=bias_d[m * 128:(m + 1) * 128]
                                                      .rearrange("(a b) -> a b", b=1))
                                    nc.vector.tensor_scalar_add(a[:, :], ps[:, :],
                                                                bt[:, :])
                                else:
                                    nc.scalar.activation(out=a[:, :], in_=ps[:, :],
                                                         func=AF.Copy)
                                acts.append(a)
                                sq = p1tmp.tile([128, R], F32R, tag="sq")
                                nc.vector.tensor_mul(sq[:, :], a[:, :], a[:, :])
                                nc.tensor.matmul(sums_ps[:, :], ones_col[:, :], sq[:, :],
                                                 start=(m == 0), stop=(m == 3),
                                                 skip_group_check=True)
                            else:       # kv rope chunk [64, R]
                                if flags['ba'][path]:
                                    bt = p1sb.tile([64, 1], F32, tag="bias4",
                                                   name="bkpe")
                                    nc.sync.dma_start(out=bt,
                                                      in_=bias_d[512:576]
                                                      .rearrange("(a b) -> a b", b=1))
                                    t1 = p1tmp.tile([64, R], F32R, tag="t1")
                                    nc.vector.scalar_tensor_tensor(
                                        out=t1[:, :], in0=ps[0:64, :], scalar=bt[:, :],
                                        in1=t1c[:, :], op0=ALU.add, op1=ALU.mult)
                                    t2 = p1tmp.tile([64, R], F32R, tag="t2")
                                    nc.vector.scalar_tensor_tensor(
                                        out=t2[:, :], in0=ps[0:64, :], scalar=bt[:, :],
                                        in1=t2c[:, :], op0=ALU.add, op1=ALU.mult)
                                else:
                                    t1 = p1tmp.tile([64, R], F32R, tag="t1")
                                    nc.vector.tensor_mul(t1[:, :], ps[0:64, :], t1c[:, :])
                                    t2 = p1tmp.tile([64, R], F32R, tag="t2")
                                    nc.vector.tensor_mul(t2[:, :], ps[0:64, :], t2c[:, :])
                                psr = p1ps2.tile([64, R], F32, tag="psr")
                                nc.tensor.matmul(psr[:, :], smat_t[0:64, 0:64],
                                                 t2[:, :], start=True, stop=True)
                                rot = p1tmp.tile([64, R], BF16, tag="rot")
                                nc.vector.tensor_add(rot[:, :], psr[:, :], t1[:, :])
                                nc.sync.dma_start(out=agkv_in[KVLR:KVLR + ROPE, :],
                                                  in_=rot[:, :])
                        # rstd = 1/sqrt(mean + eps), fold norm weight via matmul
                        std = p1tmp.tile([1, R], F32, tag="std")
                        nc.scalar.activation(out=std[:, :], in_=sums_ps[:, :],
                                             func=AF.Sqrt,
                                             scale=1.0 / (QLR if path == 0 else KVLR),
                                             bias=eps_t[:, :])
                        rstd_f = p1tmp.tile([1, R], F32, tag="rstdf")
                        nc.vector.reciprocal(out=rstd_f[:, :], in_=std[:, :])
                        rstd = p1tmp.tile([1, R], F32R, tag="rstd")
                        nc.vector.tensor_copy(out=rstd[:, :], in_=rstd_f[:, :])
                        for m in range(4):
                            wrow = p1sb.tile([1, 128], F32R, tag=f"wrow{m}",
                                             name=f"w{path}{m}")
                            nc.sync.dma_start(out=wrow,
                                              in_=normw_d[m * 128:(m + 1) * 128]
                                              .rearrange("(b a) -> b a", b=1).bitcast(F32R))
                            rep = p1ps2.tile([128, R], F32, tag="p1rep")
                            nc.tensor.matmul(rep[:, :], wrow[:, :], rstd[:, :],
                                             start=True, stop=True)
                            nrm = p1tmp.tile([128, R], BF16, tag="nrm")
                            nc.vector.tensor_mul(nrm[:, :], acts[m][:, :], rep[:, :])
                            nc.sync.dma_start(
                                out=ag_dst[m * 128:(m + 1) * 128, :],
                                in_=nrm[:, :])
                        acts.clear()
                        # kick this path's AllGather as soon as its rows land
                        ag_i, ag_o = ((agq_in, agq_out) if path == 0
                                      else (agkv_in, agkv_out))
                        if sim_mode:
                            nc.sync.dma_start(out=ag_o[0, 0:1, :], in_=ag_i[0:1, :])
                        else:
                            nc.gpsimd.collective_compute(
                                "AllGather", ALU.bypass,
                                replica_groups=[list(range(NC))],
                                ins=[ag_i.opt()], outs=[ag_o.opt()])

                # ================= PHASE 2: per-head up-projections ================
                with tc.tile_pool(name=f"p2w{_rep}", bufs=1) as p2w, \
                     tc.tile_pool(name=f"p2lat{_rep}", bufs=2) as p2lat, \
                     tc.tile_pool(name=f"p2tmp{_rep}", bufs=2) as p2tmp, \
                     tc.tile_pool(name=f"p2ps{_rep}", bufs=3, space="PSUM") as p2ps, \
                     tc.tile_pool(name=f"p2psr{_rep}", bufs=1, space="PSUM") as p2psr:

                    wkb_t = p2w.tile([128, 2, 4, 128], BF16)
                    nc.sync.dma_start(out=wkb_t, in_=wkbT[:, :, :, :])
                    wvb_t = p2w.tile([128, 4, HPC * VD], BF16)
                    nc.sync.dma_start(out=wvb_t, in_=wvbT[:, :, :])
                    if flags['bvb']:
                        bvb_t = p2w.tile([1, HPC * VD], F32R)
                        nc.sync.dma_start(out=bvb_t, in_=bvb[:, :].bitcast(F32R))
                    if flags['bkb']:
                        bk_t = [p2w.tile([128, 1], F32, name=f"bkt{m}") for m in range(2)]
                        for m in range(2):
                            nc.sync.dma_start(out=bk_t[m],
                                              in_=bkb[m * 128:(m + 1) * 128]
                                              .rearrange("(a b) -> a b", b=1))

                    # ---- pass A: k_nope / k_pe / v from the kv AllGather ----
                    for s in range(NBLK):
                        tsl = slice(s * R, (s + 1) * R)
                        kn = p2lat.tile([128, 4, R], BF16, tag="kn")
                        nc.sync.dma_start(out=kn,
                                          in_=agkv_out[s, 0:KVLR, :]
                                          .rearrange("(k p) t -> p k t", p=128))
                        kpe2 = p2lat.tile([128, R], BF16, tag="kpe")
                        nc.scalar.dma_start(out=kpe2[0:64, :],
                                            in_=agkv_out[s, KVLR:KVLR + ROPE, :])
                        nc.scalar.dma_start(out=kpe2[64:128, :],
                                            in_=agkv_out[s, KVLR:KVLR + ROPE, :])
                        for m in range(2):   # head m k_nope
                            ps = p2ps.tile([128, R], F32, tag="p2acc")
                            for k in range(4):
                                nc.tensor.matmul(ps[:, :], wkb_t[:, m, k, :],
                                                 kn[:, k, :],
                                                 start=(k == 0), stop=(k == 3))
                            if flags['bkb']:
                                nc.vector.tensor_scalar_add(kpk[m][:, 0, tsl],
                                                            ps[:, :], bk_t[m][:, :])
                            else:
                                nc.vector.tensor_copy(out=kpk[m][:, 0, tsl],
                                                      in_=ps[:, :])
                        nc.vector.tensor_copy(out=kpk[0][0:64, 1, tsl],
                                              in_=kpe2[0:64, :])
                        nc.vector.tensor_copy(out=kpk[1][64:128, 1, tsl],
                                              in_=kpe2[64:128, :])
                        for mt in range(4):  # v (token-major)
                            ps = p2ps.tile([128, HPC * VD], F32, tag="p2v")
                            if flags['bvb']:
                                nc.tensor.matmul(ps[:, :], ones_row[:, :], bvb_t[:, :],
                                                 start=True, stop=False)
                            for k in range(4):
                                nc.tensor.matmul(
                                    ps[:, :],
                                    kn[:, k, mt * 128:(mt + 1) * 128],
                                    wvb_t[:, k, :],
                                    start=(k == 0 and not flags['bvb']),
                                    stop=(k == 3))
                            nc.vector.tensor_copy(out=vt[:, s * 4 + mt, :],
                                                  in_=ps[:, :])

                    # ---- pass B: q_nope / q_rope from the q AllGather ----
                    tq1 = p2w.tile([128, S], BF16)
                    nc.scalar.dma_start(out=tq1, in_=t1q_d[:, :])
                    tq2 = p2w.tile([128, S], BF16)
                    nc.scalar.dma_start(out=tq2, in_=t2q_d[:, :])
                    wqb_t = p2w.tile([128, 3, 4, 128], BF16)
                    nc.scalar.dma_start(out=wqb_t, in_=wqbT[:, :, :, :])
                    if flags['bqb']:
                        bq_t = [p2w.tile([128, 1], F32, name=f"bqt{m}") for m in range(3)]
                        for m in range(3):
                            nc.sync.dma_start(out=bq_t[m],
                                              in_=bqb[m * 128:(m + 1) * 128]
                                              .rearrange("(a b) -> a b", b=1))

                    for s in range(NBLK):
                        tsl = slice(s * R, (s + 1) * R)
                        pos = (s % (S // R)) * R       # position within batch
                        psl = slice(pos, pos + R)
                        qn = p2lat.tile([128, 4, R], BF16, tag="qn")
                        nc.scalar.dma_start(out=qn,
                                            in_=agq_out[s, :, :]
                                            .rearrange("(k p) t -> p k t", p=128))
                        for m in range(3):
                            ps = p2ps.tile([128, R], F32, tag="p2acc")
                            for k in range(4):
                                nc.tensor.matmul(ps[:, :], wqb_t[:, m, k, :],
                                                 qn[:, k, :],
                                                 start=(k == 0), stop=(k == 3))
                            if m < 2:    # head m q_nope
                                if flags['bqb']:
                                    nc.vector.tensor_scalar_add(qpk[m][:, 0, tsl],
                                                                ps[:, :], bq_t[m][:, :])
                                else:
                                    nc.scalar.activation(out=qpk[m][:, 0, tsl],
                                                         in_=ps[:, :], func=AF.Copy)
                            else:        # rope rows for both heads, split layout
                                if flags['bqb']:
                                    t1 = p2tmp.tile([128, R], F32R, tag="t1")
                                    nc.vector.scalar_tensor_tensor(
                                        out=t1[:, :], in0=ps[:, :],
                                        scalar=bq_t[2][:, :], in1=tq1[:, psl],
                                        op0=ALU.add, op1=ALU.mult)
                                    t2 = p2tmp.tile([128, R], F32R, tag="t2")
                                    nc.vector.scalar_tensor_tensor(
                                        out=t2[:, :], in0=ps[:, :],
                                        scalar=bq_t[2][:, :], in1=tq2[:, psl],
                                        op0=ALU.add, op1=ALU.mult)
                                else:
                                    t1 = p2tmp.tile([128, R], F32R, tag="t1")
                                    nc.vector.tensor_mul(t1[:, :], ps[:, :], tq1[:, psl])
                                    t2 = p2tmp.tile([128, R], F32R, tag="t2")
                                    nc.vector.tensor_mul(t2[:, :], ps[:, :], tq2[:, psl])
                                psr = p2psr.tile([128, R], F32, tag="psr")
                                nc.tensor.matmul(psr[:, :], smat_t[:, :], t2[:, :],
                                                 start=True, stop=True)
                                nc.vector.tensor_add(qpk[0][0:64, 1, tsl],
                                                     psr[0:64, :], t1[0:64, :])
                                nc.vector.tensor_add(qpk[1][64:128, 1, tsl],
                                                     psr[64:128, :], t1[64:128, :])

                # prefetch the output-projection weights during attention
                # (Pool queue: idle there, and queued after the collectives)
                rh = [p4w.tile([128, 16, 512], BF16, name=f"rh{n_}")
                      for n_ in range(4)]
                for n_ in range(4):
                    nc.gpsimd.dma_start(out=rh[n_], in_=woT[n_, :, :, :])

                # ================= PHASE 3: attention =============================
                with tc.tile_pool(name=f"p3m{_rep}", bufs=1) as p3m, \
                     tc.tile_pool(name=f"p3p{_rep}", bufs=3) as p3p, \
                     tc.tile_pool(name=f"p3o{_rep}", bufs=3) as p3o, \
                     tc.tile_pool(name=f"p3sc{_rep}", bufs=2, space="PSUM") as p3sc, \
                     tc.tile_pool(name=f"p3out{_rep}", bufs=2, space="PSUM") as p3out, \
                     tc.tile_pool(name=f"p3rs{_rep}", bufs=2, space="PSUM") as p3rs:

                    mtiles = [p3m.tile([128, 2, 512], F8, name=f"mt{i}")
                              for i in range(nmask)]
                    for i in range(nmask):
                        nc.sync.dma_start(out=mtiles[i],
                                          in_=maskblk[i, :, :, :].bitcast(F8))

                    for b in range(B):
                        for lh in range(HPC):
                            for qc in reversed(range(S // 512)):
                                qsl = slice(b * S + qc * 512, b * S + (qc + 1) * 512)
                                out_ps = p3out.tile([128, 512], F32, tag="outp")
                                den_ps = p3rs.tile([128, 512], F32, tag="rsp")
                                kbs = [kb for kb in range(S // 128)
                                       if cls[qc][kb] != SKIP]
                                # pair consecutive k-blocks: one exp per pair
                                groups = []
                                j = 0
                                while j < len(kbs):
                                    if j + 1 < len(kbs) and kbs[j + 1] == kbs[j] + 1:
                                        groups.append((kbs[j], kbs[j + 1]))
                                        j += 2
                                    else:
                                        groups.append((kbs[j],))
                                        j += 1
                                nb = 0
                                for grp in groups:
                                    sc = p3sc.tile([128, 2, 512], F32, tag="sc")
                                    P = p3p.tile([128, 2, 512], BF16, tag="P")
                                    for u, kb in enumerate(grp):
                                        ksl = slice(b * S + kb * 128,
                                                    b * S + kb * 128 + 128)
                                        masked = cls[qc][kb] >= 0
                                        nc.tensor.matmul(sc[:, u, :],
                                                         kpk[lh][:, :, ksl],
                                                         qpk[lh][:, :, qsl],
                                                         start=True, stop=not masked,
                                                         perf_mode=DR)
                                        if masked:
                                            # additive mask via identity matmul:
                                            # sc[k,q] += sum_d I[d,k]*M[d,q]
                                            nc.tensor.matmul(
                                                sc[:, u, :], id8_t[:, :, :],
                                                mtiles[cls[qc][kb]][:, :, :],
                                                start=False, stop=True,
                                                perf_mode=DR)
                                    P8 = p3p.tile([128, 2, 512], F8, tag="P8")
                                    if len(grp) == 2:
                                        nc.scalar.activation(out=P[:, :, :],
                                                             in_=sc[:, :, :],
                                                             func=AF.Exp, scale=SCALE)
                                        # fp8 copy feeds the (error-cancelling)
                                        # denominator only; numerator stays bf16
                                        nc.gpsimd.tensor_copy(out=P8[:, :, :],
                                                              in_=P[:, :, :])
                                    else:
                                        nc.scalar.activation(out=P[:, 0, :],
                                                             in_=sc[:, 0, :],
                                                             func=AF.Exp, scale=SCALE)
                                        nc.gpsimd.tensor_copy(out=P8[:, 0, :],
                                                              in_=P[:, 0, :])
                                    first, nb0 = (nb == 0), nb
                                    for u, kb in enumerate(grp):
                                        nb += 1
                                        last = (nb == len(kbs))
                                        nc.tensor.matmul(
                                            out_ps[:, :],
                                            vt[:, b * 16 + kb, lh * VD:(lh + 1) * VD],
                                            P[:, u, :], start=(nb == 1), stop=last,
                                            skip_group_check=True)
                                    if len(grp) == 2:
                                        nc.tensor.matmul(den_ps[:, :], ones8[:, :, :],
                                                         P8[:, :, :], start=first,
                                                         stop=(nb == len(kbs)),
                                                         perf_mode=DR,
                                                         skip_group_check=True)
                                    else:
                                        nc.tensor.matmul(den_ps[:, :], ones8[:, 0, :],
                                                         P8[:, 0, :], start=first,
                                                         stop=(nb == len(kbs)),
                                                         skip_group_check=True)
                                # den_ps rows all hold the same denominator
                                inv_sb = p3o.tile([128, 512], F32, tag="invf")
                                nc.vector.reciprocal(out=inv_sb[:, :], in_=den_ps[:, :])
                                ao = p3o.tile([128, 512], BF16, tag="ao")
                                nc.vector.tensor_mul(ao[:, :], out_ps[:, :], inv_sb[:, :])
                                nc.sync.dma_start(
                                    out=a2a_in[b * 4 + qc, lh * VD:(lh + 1) * VD, :],
                                    in_=ao[:, :])

                qkv.release()

                # ---- AllToAll ----
                if sim_mode:
                    nc.sync.dma_start(out=a2a_out[0, 0:1, :], in_=a2a_in[0, 0:1, :])
                else:
                    nc.gpsimd.collective_compute(
                        "AllToAll", ALU.bypass,
                        replica_groups=[list(range(NC))],
                        ins=[a2a_in.opt()], outs=[a2a_out.opt()])

                # ================= PHASE 4: output projection =====================
                with tc.tile_pool(name=f"p4l{_rep}", bufs=1) as p4l, \
                     tc.tile_pool(name=f"p4o{_rep}", bufs=3) as p4o, \
                     tc.tile_pool(name=f"p4ps{_rep}", bufs=4, space="PSUM") as p4ps:

                    lt = p4l.tile([128, 16, 512], BF16)
                    av = a2a_out[:, :, :].rearrange("c (h p) t -> (c h) p t", h=2)
                    for k in range(16):
                        nc.sync.dma_start(out=lt[:, k, :], in_=av[k, :, :])
                    if flags['wob']:
                        wob_t = p4l.tile([1, DIM], F32R)
                        nc.sync.dma_start(out=wob_t, in_=wob[:, :].bitcast(F32R))
                    for n_ in range(4):
                        for m in range(4):
                            ps = p4ps.tile([128, 512], F32, tag="p4acc")
                            if flags['wob']:
                                nc.tensor.matmul(ps[:, :], ones_row[:, :],
                                                 wob_t[:, n_ * 512:(n_ + 1) * 512],
                                                 start=True, stop=False)
                            for k in range(16):
                                nc.tensor.matmul(ps[:, :],
                                                 lt[:, k, m * 128:(m + 1) * 128],
                                                 rh[n_][:, k, :],
                                                 start=(k == 0 and not flags['wob']),
                                                 stop=(k == 15))
                            ob = p4o.tile([128, 512], F32, tag="ob")
                            nc.scalar.activation(out=ob[:, :], in_=ps[:, :], func=AF.Copy)
                            nc.sync.dma_start(
                                out=out_c[m * 128:(m + 1) * 128,
                                          n_ * 512:(n_ + 1) * 512],
                                in_=ob[:, :])
                p4w.release()

    nc.finalize()
    return nc


_ROPE_PERM = np.concatenate([np.arange(0, ROPE, 2), np.arange(1, ROPE, 2)])

_CACHE = {}


def _prep_inputs(inputs):
    """Host-side slicing/permutation -> (schedule key data, per-core in_maps)."""
    x = np.ascontiguousarray(np.asarray(inputs['x'], np.float32).reshape(N, DIM))
    mask = np.asarray(inputs['mask'])
    cls, blocks = _classify_mask(mask)

    cos_t, sin_t = _rope_tables()            # [32, S]
    t1q = np.concatenate([cos_t, cos_t, cos_t, cos_t], 0).astype(BF)   # [128, S]
    t2q = np.concatenate([sin_t, -sin_t, sin_t, -sin_t], 0).astype(BF)
    s64 = np.zeros((64, 64), np.float32)
    s64[0:32, 32:64] = np.eye(32)
    s64[32:64, 0:32] = np.eye(32)
    smat = np.zeros((128, 128), np.float32)
    smat[0:64, 0:64] = s64
    smat[64:128, 64:128] = s64

    wq_a = np.asarray(inputs['wq_a_w'], np.float32)            # [QLR, DIM]
    wkv_a = np.asarray(inputs['wkv_a_w'], np.float32)          # [KVLR+ROPE, DIM]
    wkv_a_p = np.concatenate([wkv_a[:KVLR], wkv_a[KVLR:][_ROPE_PERM]], 0)
    bkva = np.asarray(inputs['wkv_a_b'], np.float32)
    bkva_p = np.concatenate([bkva[:KVLR], bkva[KVLR:][_ROPE_PERM]], 0)

    wq_b = np.asarray(inputs['wq_b_w'], np.float32).reshape(H, QKD, QLR)
    bq_b = np.asarray(inputs['wq_b_b'], np.float32).reshape(H, QKD)
    wkv_b = np.asarray(inputs['wkv_b_w'], np.float32).reshape(H, NOPE + VD, KVLR)
    bkv_b = np.asarray(inputs['wkv_b_b'], np.float32).reshape(H, NOPE + VD)
    wo = np.asarray(inputs['wo_w'], np.float32)                # [DIM, H*VD]

    def blk1(w2d, nm):
        # [out, DIM] -> [nm, 128(part=x chunk), 16(k), 128(out cols)]
        t = np.ascontiguousarray(w2d.T)                        # [DIM, out]
        return np.ascontiguousarray(
            t.reshape(DIM // 128, 128, nm, 128).transpose(2, 1, 0, 3)).astype(BF)

    def blk2(w2d, nm):
        # [out(nm*128), 512] -> [128(part=latent chunk), nm, 4(k), 128]
        t = np.ascontiguousarray(w2d.T)                        # [512, out]
        return np.ascontiguousarray(
            t.reshape(4, 128, nm, 128).transpose(1, 2, 0, 3)).astype(BF)

    wkva_pad = np.concatenate([wkv_a_p, np.zeros((64, DIM), np.float32)], 0)
    woT = np.ascontiguousarray(
        wo.T.reshape(16, 128, 4, 512).transpose(2, 1, 0, 3)).astype(BF)
    id8 = np.zeros((128, 2, 128), ml_dtypes.float8_e4m3fn)
    id8[:, 0, :] = np.eye(128)

    shared = {
        'id8': id8.view(np.uint8),
        'wqaT': blk1(wq_a, 4),
        'bqa': np.asarray(inputs['wq_a_b'], np.float32),
        'wkvaT': blk1(wkva_pad, 5),
        'bkva': bkva_p,
        'qnw': np.asarray(inputs['q_norm_w'], np.float32),
        'kvnw': np.asarray(inputs['kv_norm_w'], np.float32),
        't1q': t1q,
        't2q': t2q,
        'smat': smat,
        'woT': woT,
        'wob': np.asarray(inputs['wo_b'], np.float32)[None, :],
        'maskblk': blocks,
    }

    in_maps = []
    for c in range(NC):
        h0, h1 = 2 * c, 2 * c + 1
        # q_b rows: h0 nope, h1 nope, [h0 rope-e, h0 rope-o, h1 rope-e, h1 rope-o]
        wqb_c = np.concatenate([
            wq_b[h0, :NOPE], wq_b[h1, :NOPE],
            wq_b[h0, NOPE:][_ROPE_PERM], wq_b[h1, NOPE:][_ROPE_PERM]], 0)
        bqb_c = np.concatenate([
            bq_b[h0, :NOPE], bq_b[h1, :NOPE],
            bq_b[h0, NOPE:][_ROPE_PERM], bq_b[h1, NOPE:][_ROPE_PERM]], 0)
        wkb_c = np.concatenate([wkv_b[h0, :NOPE], wkv_b[h1, :NOPE]], 0)
        bkb_c = np.concatenate([bkv_b[h0, :NOPE], bkv_b[h1, :NOPE]], 0)
        wvb_c = np.concatenate([wkv_b[h0, NOPE:], wkv_b[h1, NOPE:]], 0)
        bvb_c = np.concatenate([bkv_b[h0, NOPE:], bkv_b[h1, NOPE:]], 0)
        pos = (c % (S // R)) * R
        cos_c, sin_c = cos_t[:, pos:pos + R], sin_t[:, pos:pos + R]
        m = dict(shared)
        m.update({
            'x_c': np.ascontiguousarray(
                x[c * R:(c + 1) * R].T.reshape(DIM // 128, 128, R)
                .transpose(1, 0, 2)).astype(BF),
            'wqbT': blk2(wqb_c, 3),
            'bqb': bqb_c,
            'wkbT': blk2(wkb_c, 2),
            'bkb': bkb_c,
            'wvbT': np.ascontiguousarray(
                wvb_c.T.reshape(4, 128, HPC * VD).transpose(1, 0, 2)).astype(BF),
            'bvb': bvb_c[None, :],
            't1c': np.ascontiguousarray(
                np.concatenate([cos_c, cos_c], 0)).astype(BF),
            't2c': np.ascontiguousarray(
                np.concatenate([sin_c, -sin_c], 0)).astype(BF),
        })
        in_maps.append(m)
    return cls, in_maps


class _Runner:
    """Compile once, execute many times on the 8 axon-tunneled NeuronCores."""

    def __init__(self, nc):
        import jax
        from jax.experimental.shard_map import shard_map
        from jax.sharding import Mesh, PartitionSpec
        from concourse import bass2jax, mybir as _mybir
        bass2jax.install_neuronx_cc_hook()
        self.jax = jax
        in_names, out_names, out_avals, zero_outs = [], [], [], []
        partition_name = (nc.partition_id_tensor.name
                          if nc.partition_id_tensor else None)
        for alloc in nc.m.functions[0].allocations:
            if not isinstance(alloc, _mybir.MemoryLocationSet):
                continue
            name = alloc.memorylocations[0].name
            if alloc.kind == "ExternalInput":
                if name != partition_name:
                    in_names.append(name)
            elif alloc.kind == "ExternalOutput":
                shape = tuple(alloc.tensor_shape)
                dtype = _mybir.dt.np(alloc.dtype)
                out_names.append(name)
                out_avals.append(jax.core.ShapedArray(shape, dtype))
                zero_outs.append(np.zeros(shape, dtype))
        self.n_params = len(in_names)
        self.in_names = list(in_names)
        self.out_names = out_names
        self.out_avals = out_avals
        self.zero_outs = zero_outs
        all_in = in_names + out_names
        if partition_name is not None:
            all_in.append(partition_name)

        def _body(*args):
            operands = list(args)
            if partition_name is not None:
                operands.append(bass2jax.partition_id_tensor())
            outs = bass2jax._bass_exec_p.bind(
                *operands,
                out_avals=tuple(out_avals),
                in_names=tuple(all_in),
                out_names=tuple(out_names),
                lowering_input_output_aliases=(),
                sim_require_finite=True,
                sim_require_nnan=True,
                nc=nc)
            return tuple(outs)

        devices = jax.devices()[:NC]
        self.mesh = Mesh(np.asarray(devices), ("core",))
        n_out = len(out_names)
        in_specs = (PartitionSpec("core"),) * (self.n_params + n_out)
        out_specs = (PartitionSpec("core"),) * n_out
        donate = tuple(range(self.n_params, self.n_params + n_out))
        self.fn = jax.jit(
            shard_map(_body, mesh=self.mesh, in_specs=in_specs,
                      out_specs=out_specs, check_rep=False),
            donate_argnums=donate, keep_unused=True)

    def concat_inputs(self, in_maps):
        return [np.concatenate([np.asarray(in_maps[c][nm])
                                for c in range(NC)], axis=0)
                for nm in self.in_names]

    def zeros(self):
        return [np.zeros((NC * z.shape[0], *z.shape[1:]), z.dtype)
                for z in self.zero_outs]

    def __call__(self, concat_in, concat_zeros):
        out = self.fn(*concat_in, *concat_zeros)
        return out

    def run(self, in_maps):
        outs = self(self.concat_inputs(in_maps), self.zeros())
        res = []
        for c in range(NC):
            res.append({nm: np.asarray(outs[i]).reshape(NC, *self.out_avals[i].shape)[c]
                        for i, nm in enumerate(self.out_names)})
        return res


def _get_exec(cls, nmask, flags):
    key = (tuple(tuple(r) for r in cls), nmask,
           tuple(flags['ba']), flags['bqb'], flags['bkb'], flags['bvb'],
           flags['wob'])
    if key not in _CACHE:
        nc = _build(cls, nmask, flags)
        _CACHE[key] = _Runner(nc)
    return _CACHE[key]


def kernel(**inputs):
    cls, in_maps = _prep_inputs(inputs)
    nmask = max(len(in_maps[0]['maskblk']), 1)
    flags = {
        'ba': (bool(np.any(inputs['wq_a_b'])), bool(np.any(inputs['wkv_a_b']))),
        'bqb': bool(np.any(inputs['wq_b_b'])),
        'bkb': bool(np.any(np.asarray(inputs['wkv_b_b']).reshape(H, NOPE + VD)[:, :NOPE])),
        'bvb': bool(np.any(np.asarray(inputs['wkv_b_b']).reshape(H, NOPE + VD)[:, NOPE:])),
        'wob': bool(np.any(inputs['wo_b'])),
    }
    runner = _get_exec(cls, nmask, flags)
    results = runner.run(in_maps)
    out = np.concatenate([results[c]["out"] for c in range(NC)], 0)
    return out.reshape(B, S, DIM)
